# revision 8
# baseline (speedup 1.0000x reference)
"""MoFE (mixture of depthwise-conv experts) Trainium2 kernel.

Full inputs in, full outputs out; internally sharded data-parallel over the
batch dim across 8 NeuronCores (B=8, one sample per core).

Per-core program (Bass/Tile):
  pass A: stream x, per-strip max/sum reduces -> gate (pooled -> fc -> noisy
          top-k softmax coefficients, all on device)
  pass B: per strip: depthwise conv1 (+bias, relu), depthwise conv2,
          cof-weighted accumulation over experts, store.
"""

import numpy as np

import concourse.bass as bass
import concourse.tile as tile
from concourse import mybir
from concourse.bass_utils import run_bass_kernel_spmd

F32 = mybir.dt.float32
AX = mybir.AxisListType if hasattr(mybir, "AxisListType") else None
ALU = mybir.AluOpType
ACT = mybir.ActivationFunctionType

B, C, H, W = 8, 96, 192, 192
E = 6
N_CORES = 8
TH = 24                      # strip height (output rows per strip)
NS = H // TH                 # strips
TAPS = [(ky - 1, kx - 1) for ky in range(3) for kx in range(3)]


# ---------------------------------------------------------------------------
# walrus workaround: split instructions carrying >maxw semaphore waits
# ---------------------------------------------------------------------------
def _split_multiwait(nc, maxw: int = 1) -> int:
    n_split = 0
    for f in nc.m.functions:
        for b in f.blocks:
            insts = b.instructions
            new_list = []
            changed = False
            for inst in insts:
                si = getattr(inst, "sync_info", None)
                waits = list(si.on_wait) if (si and si.on_wait) else []
                if len(waits) > maxw:
                    changed = True
                    chunks = [waits[j: j + maxw] for j in range(0, len(waits), maxw)]
                    for k, ch in enumerate(chunks[:-1]):
                        nop = mybir.InstNoOp(
                            name=f"{inst.name}_wsplit{k}",
                            sync_info=mybir.SyncInfo(on_wait=ch, on_update=[]),
                            bass_nofuse=True,
                            engine=inst.engine,
                        )
                        new_list.append(nop)
                        n_split += 1
                    si.on_wait = chunks[-1]
                new_list.append(inst)
            if changed:
                if isinstance(insts, list):
                    insts[:] = new_list
                else:
                    b.instructions = new_list
    return n_split


# ---------------------------------------------------------------------------
# device program
# ---------------------------------------------------------------------------
def _build():
    nc = bass.Bass()
    x = nc.declare_dram_parameter("x", [C, H, W], F32, isOutput=False)
    wfc = nc.declare_dram_parameter("wfc", [C, 2 * E], F32, isOutput=False)
    bfc = nc.declare_dram_parameter("bfc", [1, 2 * E], F32, isOutput=False)
    w1 = nc.declare_dram_parameter("w1", [C, E * 9], F32, isOutput=False)
    b1 = nc.declare_dram_parameter("b1", [C, E], F32, isOutput=False)
    w2 = nc.declare_dram_parameter("w2", [C, E * 9], F32, isOutput=False)
    b2 = nc.declare_dram_parameter("b2", [C, E], F32, isOutput=False)
    y = nc.declare_dram_parameter("y", [C, H, W], F32, isOutput=True)

    v = nc.vector
    g = nc.gpsimd
    sc = nc.scalar
    sy = nc.sync

    with tile.TileContext(nc) as tc:
        with (
            tc.tile_pool(name="const", bufs=1) as cpool,
            tc.tile_pool(name="gate", bufs=1) as gpool,
            tc.tile_pool(name="xa", bufs=2) as xa_pool,
            tc.tile_pool(name="xp", bufs=2) as xp_pool,
            tc.tile_pool(name="hbuf", bufs=2) as h_pool,
            tc.tile_pool(name="o2", bufs=2) as o2_pool,
            tc.tile_pool(name="oacc", bufs=2) as oacc_pool,
            tc.tile_pool(name="ps", bufs=1, space="PSUM") as ps_pool,
        ):
            # ---- constants ------------------------------------------------
            w1_sb = cpool.tile([C, E * 9], F32)
            sy.dma_start(w1_sb[:], w1[:])
            b1_sb = cpool.tile([C, E], F32)
            sy.dma_start(b1_sb[:], b1[:])
            w2_sb = cpool.tile([C, E * 9], F32)
            sy.dma_start(w2_sb[:], w2[:])
            b2_sb = cpool.tile([C, E], F32)
            sy.dma_start(b2_sb[:], b2[:])
            wfc_sb = cpool.tile([C, 2 * E], F32)
            sy.dma_start(wfc_sb[:], wfc[:])
            bfc_sb = cpool.tile([1, 2 * E], F32)
            sy.dma_start(bfc_sb[:], bfc[:])

            # ---- pass A: pooled = max_hw(x) + mean_hw(x) ------------------
            maxbuf = gpool.tile([C, NS], F32)
            sumbuf = gpool.tile([C, NS], F32)
            for s in range(NS):
                xa = xa_pool.tile([C, TH, W], F32)
                sy.dma_start(xa[:], x[:, s * TH:(s + 1) * TH, :])
                v.tensor_reduce(maxbuf[:, s:s + 1], xa[:], AX.XY, ALU.max)
                v.tensor_reduce(sumbuf[:, s:s + 1], xa[:], AX.XY, ALU.add)
            maxv = gpool.tile([C, 1], F32)
            v.tensor_reduce(maxv[:], maxbuf[:], AX.X, ALU.max)
            sumv = gpool.tile([C, 1], F32)
            v.tensor_reduce(sumv[:], sumbuf[:], AX.X, ALU.add)
            pooled = gpool.tile([C, 1], F32)
            v.scalar_tensor_tensor(
                pooled[:], sumv[:], 1.0 / (H * W), maxv[:], ALU.mult, ALU.add
            )

            # ---- gate -----------------------------------------------------
            psg = ps_pool.tile([2 * E, 1], F32)
            nc.tensor.matmul(psg[:], wfc_sb[:], pooled[:], start=True, stop=True)
            g12 = gpool.tile([2 * E, 1], F32)
            v.tensor_copy(g12[:], psg[:])
            grow = gpool.tile([1, 2 * E], F32)
            sy.dma_start(grow[:], g12[:])          # partition -> free transpose
            gb = gpool.tile([1, 2 * E], F32)
            v.tensor_add(gb[:], grow[:], bfc_sb[:])
            g_pre = gb[:, 0:E]
            n_pre = gb[:, E:2 * E]

            # leaky relu(0.2)
            gl = gpool.tile([1, E], F32)
            t6 = gpool.tile([1, E], F32)
            v.tensor_scalar_mul(t6[:], g_pre, 0.2)
            v.tensor_max(gl[:], g_pre, t6[:])
            # softplus(x) = ln(1 + exp(x))  (Softplus has no ACT table here)
            e1 = gpool.tile([1, E], F32)
            sc.activation(e1[:], n_pre, ACT.Exp)
            noise = gpool.tile([1, E], F32)
            sc.activation(noise[:], e1[:], ACT.Ln, bias=1.0)
            # mean / unbiased std over experts
            mu = gpool.tile([1, 1], F32)
            v.tensor_reduce(mu[:], noise[:], AX.X, ALU.add)
            v.tensor_scalar_mul(mu[:], mu[:], 1.0 / E)
            d = gpool.tile([1, E], F32)
            v.tensor_scalar(d[:], noise[:], mu[:], None, ALU.subtract)
            dd = gpool.tile([1, E], F32)
            v.tensor_mul(dd[:], d[:], d[:])
            var = gpool.tile([1, 1], F32)
            v.tensor_reduce(var[:], dd[:], AX.X, ALU.add)
            v.tensor_scalar_mul(var[:], var[:], 1.0 / (E - 1))
            # 1/sqrt(var) via exp(-0.5 ln var) + one Newton step
            lnv = gpool.tile([1, 1], F32)
            sc.activation(lnv[:], var[:], ACT.Ln)
            isd0 = gpool.tile([1, 1], F32)
            sc.activation(isd0[:], lnv[:], ACT.Exp, scale=-0.5)
            ii = gpool.tile([1, 1], F32)
            v.tensor_mul(ii[:], isd0[:], isd0[:])
            v.tensor_mul(ii[:], ii[:], var[:])
            v.tensor_scalar(ii[:], ii[:], -0.5, 1.5, ALU.mult, ALU.add)
            isd = gpool.tile([1, 1], F32)
            v.tensor_mul(isd[:], isd0[:], ii[:])
            scores = gpool.tile([1, E], F32)
            v.scalar_tensor_tensor(scores[:], d[:], isd[:], gl[:], ALU.mult, ALU.add)

            # rank each expert, mask = rank < 3
            ranks = gpool.tile([1, E], F32)
            cmp = gpool.tile([1, E], F32)
            for e in range(E):
                v.tensor_scalar(
                    cmp[:], scores[:], scores[0:1, e:e + 1], None, ALU.is_gt
                )
                v.tensor_reduce(ranks[:, e:e + 1], cmp[:], AX.X, ALU.add)
            mask = gpool.tile([1, E], F32)
            v.tensor_scalar(mask[:], ranks[:], 3.0, None, ALU.is_lt)

            # softmax over selected: gm = (gl+30)*mask - 30
            gm = gpool.tile([1, E], F32)
            v.scalar_tensor_tensor(gm[:], gl[:], 30.0, mask[:], ALU.add, ALU.mult)
            v.tensor_scalar_sub(gm[:], gm[:], 30.0)
            gmax = gpool.tile([1, 1], F32)
            v.tensor_reduce(gmax[:], gm[:], AX.X, ALU.max)
            ngmax = gpool.tile([1, 1], F32)
            v.tensor_scalar_mul(ngmax[:], gmax[:], -1.0)
            ex = gpool.tile([1, E], F32)
            sc.activation(ex[:], gm[:], ACT.Exp, bias=ngmax[:])
            ssum = gpool.tile([1, 1], F32)
            v.tensor_reduce(ssum[:], ex[:], AX.X, ALU.add)
            rs = gpool.tile([1, 1], F32)
            v.reciprocal(rs[:], ssum[:])
            cof = gpool.tile([1, E], F32)
            v.tensor_scalar(cof[:], ex[:], rs[:], None, ALU.mult)

            # broadcast cof to all partitions: [96,6] = ones[1,96].T @ cof[1,6]
            ones96 = cpool.tile([1, C], F32)
            g.memset(ones96[:], 1.0)
            ps_cof = ps_pool.tile([C, E], F32)
            nc.tensor.matmul(ps_cof[:], ones96[:], cof[:], start=True, stop=True)
            cof_b = cpool.tile([C, E], F32)
            v.tensor_copy(cof_b[:], ps_cof[:])
            # b2tot = sum_e cof_e * b2_e
            tb = gpool.tile([C, E], F32)
            v.tensor_mul(tb[:], b2_sb[:], cof_b[:])
            b2tot = cpool.tile([C, 1], F32)
            v.tensor_reduce(b2tot[:], tb[:], AX.X, ALU.add)

            # ---- pass B: experts ------------------------------------------
            WP = W + 2
            for s in range(NS):
                h0 = s * TH
                xp = xp_pool.tile([C, TH + 4, WP], F32)
                g.memset(xp[:, :, 0:1], 0.0)
                g.memset(xp[:, :, WP - 1:WP], 0.0)
                lo = max(0, h0 - 2)
                hi = min(H, h0 + TH + 2)
                off = lo - (h0 - 2)
                if off > 0:
                    g.memset(xp[:, 0:off, :], 0.0)
                if hi - lo < TH + 4 - off:
                    g.memset(xp[:, off + (hi - lo):TH + 4, :], 0.0)
                sy.dma_start(xp[:, off:off + (hi - lo), 1:W + 1], x[:, lo:hi, :])

                oacc = oacc_pool.tile([C, TH, W], F32)
                for e in range(E):
                    ht = h_pool.tile([C, TH + 2, WP], F32)
                    g.memset(ht[:, :, 0:1], 0.0)
                    g.memset(ht[:, :, WP - 1:WP], 0.0)
                    hv = ht[:, :, 1:W + 1]          # [C, TH+2, W]
                    for it, (dy, dx) in enumerate(TAPS):
                        in0 = xp[:, 1 + dy:1 + dy + TH + 2, 1 + dx:1 + dx + W]
                        wap = w1_sb[:, e * 9 + it:e * 9 + it + 1]
                        if it == 0:
                            v.tensor_scalar(hv, in0, wap, None, ALU.mult)
                        else:
                            v.scalar_tensor_tensor(hv, in0, wap, hv, ALU.mult, ALU.add)
                    sc.activation(hv, hv, ACT.Relu, bias=b1_sb[:, e:e + 1])
                    if s == 0:
                        g.memset(ht[:, 0:1, :], 0.0)
                    if s == NS - 1:
                        g.memset(ht[:, TH + 1:TH + 2, :], 0.0)

                    o2 = o2_pool.tile([C, TH, W], F32)
                    for it, (dy, dx) in enumerate(TAPS):
                        in0 = ht[:, 1 + dy:1 + dy + TH, 1 + dx:1 + dx + W]
                        wap = w2_sb[:, e * 9 + it:e * 9 + it + 1]
                        if it == 0:
                            v.tensor_scalar(o2[:], in0, wap, None, ALU.mult)
                        else:
                            v.scalar_tensor_tensor(o2[:], in0, wap, o2[:], ALU.mult, ALU.add)
                    cap = cof_b[:, e:e + 1]
                    if e == 0:
                        v.tensor_scalar(oacc[:], o2[:], cap, None, ALU.mult)
                    else:
                        v.scalar_tensor_tensor(oacc[:], o2[:], cap, oacc[:], ALU.mult, ALU.add)
                v.tensor_scalar(oacc[:], oacc[:], b2tot[:], None, ALU.add)
                sy.dma_start(y[:, h0:h0 + TH, :], oacc[:])

    _split_multiwait(nc, maxw=1)
    return nc


_NC_CACHE = {}


def _get_nc():
    if "nc" not in _NC_CACHE:
        _NC_CACHE["nc"] = _build()
    return _NC_CACHE["nc"]


class _Runner:
    """Compile-once SPMD runner (mirrors bass2jax.run_bass_via_pjrt's
    multi-core path, but keeps the jitted executable for reuse/benching)."""

    def __init__(self, nc, n_cores):
        import jax
        from jax.experimental.shard_map import shard_map
        from jax.sharding import Mesh, PartitionSpec
        from concourse import bass2jax, mybir as _mybir

        bass2jax.install_neuronx_cc_hook()
        self.jax = jax
        partition_name = (
            nc.partition_id_tensor.name if nc.partition_id_tensor else None
        )
        in_names, out_names, out_avals, zero_outs = [], [], [], []
        for alloc in nc.m.functions[0].allocations:
            if not isinstance(alloc, _mybir.MemoryLocationSet):
                continue
            name = alloc.memorylocations[0].name
            if alloc.kind == "ExternalInput":
                if name == partition_name:
                    continue
                in_names.append(name)
            elif alloc.kind == "ExternalOutput":
                shape = tuple(alloc.tensor_shape)
                dtype = _mybir.dt.np(alloc.dtype)
                out_names.append(name)
                out_avals.append(jax.core.ShapedArray(shape, dtype))
                zero_outs.append(np.zeros(shape, dtype))
        self.in_names, self.out_names = in_names, out_names
        self.out_avals, self.zero_outs = out_avals, zero_outs
        n_params, n_outs = len(in_names), len(out_names)
        self.n_cores = n_cores
        donate = tuple(range(n_params, n_params + n_outs))

        all_in_names = in_names + out_names
        if partition_name is not None:
            all_in_names = all_in_names + [partition_name]

        def _body(*args):
            operands = list(args)
            if partition_name is not None:
                operands.append(bass2jax.partition_id_tensor())
            outs = bass2jax._bass_exec_p.bind(
                *operands,
                out_avals=tuple(out_avals),
                in_names=tuple(all_in_names),
                out_names=tuple(out_names),
                lowering_input_output_aliases=(),
                sim_require_finite=True,
                sim_require_nnan=True,
                nc=nc,
            )
            return tuple(outs)

        devices = jax.devices()[:n_cores]
        mesh = Mesh(np.asarray(devices), ("core",))
        self.sharded = jax.jit(
            shard_map(
                _body,
                mesh=mesh,
                in_specs=(PartitionSpec("core"),) * (n_params + n_outs),
                out_specs=(PartitionSpec("core"),) * n_outs,
                check_rep=False,
            ),
            donate_argnums=donate,
            keep_unused=True,
        )

    def concat_inputs(self, in_maps):
        return [
            np.concatenate([np.asarray(m[name]) for m in in_maps], axis=0)
            for name in self.in_names
        ]

    def concat_zeros(self):
        return [
            np.zeros((self.n_cores * z.shape[0], *z.shape[1:]), z.dtype)
            for z in self.zero_outs
        ]

    def run(self, in_maps):
        out_arrs = self.sharded(*self.concat_inputs(in_maps), *self.concat_zeros())
        return [
            {
                name: np.asarray(out_arrs[i]).reshape(
                    self.n_cores, *self.out_avals[i].shape
                )[c]
                for i, name in enumerate(self.out_names)
            }
            for c in range(self.n_cores)
        ]


def _get_runner():
    if "runner" not in _NC_CACHE:
        _NC_CACHE["runner"] = _Runner(_get_nc(), N_CORES)
    return _NC_CACHE["runner"]


def kernel(x, w_fc0, b_fc0, w_fc1, b_fc1, ew1, eb1, ew2, eb2):
    x = np.asarray(x, dtype=np.float32)
    f32 = lambda a: np.ascontiguousarray(np.asarray(a, dtype=np.float32))
    wfc = f32(np.concatenate([np.asarray(w_fc1).T, np.asarray(w_fc0).T], axis=1))
    bfc = f32(np.concatenate([np.asarray(b_fc1), np.asarray(b_fc0)])[None, :])
    w1p = f32(np.asarray(ew1).reshape(E, C, 9).transpose(1, 0, 2).reshape(C, E * 9))
    b1p = f32(np.asarray(eb1).T)
    w2p = f32(np.asarray(ew2).reshape(E, C, 9).transpose(1, 0, 2).reshape(C, E * 9))
    b2p = f32(np.asarray(eb2).T)

    in_maps = []
    for b in range(B):
        in_maps.append({
            "x": np.ascontiguousarray(x[b]),
            "wfc": wfc, "bfc": bfc,
            "w1": w1p, "b1": b1p, "w2": w2p, "b2": b2p,
        })
    res = _get_runner().run(in_maps)
    out = np.stack([res[b]["y"] for b in range(B)], axis=0)
    return out.astype(np.float32)


if __name__ == "__main__":
    data = np.load("/tmp/ref_data.npz")
    inputs = {k: data[k] for k in
              ["x", "w_fc0", "b_fc0", "w_fc1", "b_fc1", "ew1", "eb1", "ew2", "eb2"]}
    out = kernel(**inputs)
    exp = data["out"]
    err = np.linalg.norm(out - exp) / np.linalg.norm(exp)
    print("Relative error:", err)
    print("max abs diff:", np.abs(out - exp).max())
